# revision 13
# baseline (speedup 1.0000x reference)
"""nn_Attention_60266981097535 — Bass/Tile kernel for 8 trn2 NeuronCores.

Sharding: 8 cores = (batch b in 0..3) x (query-row half in 0..1), data-parallel
SPMD. Per-core inputs are host-permuted so each core's rows are always [0:512]
(row+column permutation of adj, matching row permutation of x; softmax and all
contractions are permutation-covariant, output rows are written back to the
original slice).

Device pipeline per core (all big matmuls bf16, accumulation f32 in PSUM):
  x -> xT -> xq|xk|xv (block-diag weight matmul)
  GCN: adjT shipped BIT-PACKED (u8 bit-planes, 1MB vs 32MB i32); DVE unpack
       (pk>>j)&1 + u8->bf16 copy straight into the transposed lhsT layout
       q_g = relu(adj @ xq), k_g likewise (full rows, V1 collective-free)
  q = q_g @ Wq, k = k_g @ Wk  (kept transposed: qT [dim, tok])
  R0 = gelu((q|k) @ Wkf) via sigmoid-approx gelu; Gram = R0^T R0
  R = sigmoid((Gram @ Wkf2) / sparse_D); q' = qT * R_exp * scale; k' = kT * R_exp
  attn logits: premix[h] = lrelu(q'^T k') via DVE max(0.01z, z)
  head mixing: SBUF->SBUF DMA repack to (h,ss)-partition packing, then one
       block-diagonal randomatrix matmul on PE (K=128)
  softmax: e = exp(mixed) (logits are small, no max-sub needed; verified),
       n = e * adj_mask (fp8 mask input), S = rowsum (fused DVE stt)
  v: nT = n^T scaled by 1/S via matmul against diag(1/S); v = relu(xv^T @ nT)
  out = gelu(v @ Wv), transposed back and DMA'd out.
"""

import numpy as np

B, T, DIM, H, D = 4, 1024, 256, 8, 32
HALF = T // 2
N_CORES = 8

_CACHE = {}


def _patch_tail_drain():
    """Split the TileContext tail drain's sem waits: the walrus build in this
    environment rejects >2 sync-wait commands per CTRL instruction."""
    import concourse.tile as tile
    import concourse.tile_sem_assignment as tsa
    from concourse.vector_clock import VectorClock, ScopedClock

    # fewer DMA completion lanes -> fewer distinct sems per waiting instruction
    tsa.NUM_HWDGE_SEMS = 2

    if getattr(tile.TileContext, "_ant_split_drain", False):
        return

    def _split_drain_and_barrier(self, tick_clock, wait_clock):
        vec = tick_clock.global_clock
        n = len(vec)
        procs = [i for i in range(n) if vec[i] > 0]
        for i in range(0, len(procs), 2):
            chunk = set(procs[i:i + 2])
            v2 = VectorClock([vec[j] if j in chunk else 0 for j in range(n)])
            d = self.nc.sync.drain()
            wait_clock.add_sem_waits(d.ins, ScopedClock({None: v2}))
        self.nc.all_engine_barrier()
        assert self.sems is not None
        popped = self.nc._tile_sem_poison_stack.pop()
        assert popped is self._sem_poison
        self.nc.clear_and_free_semaphores(list(self.sems.allocated().values()))
        self.nc.all_engine_barrier()

    tile.TileContext._drain_and_barrier = _split_drain_and_barrier
    tile.TileContext._ant_split_drain = True


def _split_waits(nc, mybir, limit=1):
    """This walrus build encodes at most one sync-wait per instruction.
    Move excess waits onto same-engine NoOp carriers inserted just before."""
    nid = 0
    for blk in nc.m.functions[0].blocks:
        out, changed = [], False
        for inst in blk.instructions:
            si = inst.sync_info
            waits = list(si.on_wait) if (si is not None and si.on_wait) else []
            if len(waits) > limit:
                head, keep = waits[:-limit], waits[-limit:]
                for i in range(0, len(head), limit):
                    nop = mybir.InstNoOp(name=f"WS-{nid}", ins=[], outs=[])
                    nid += 1
                    nop.engine = inst.engine
                    nop.sync_info = mybir.SyncInfo(on_wait=head[i:i + limit],
                                                   on_update=[])
                    out.append(nop)
                inst.sync_info = mybir.SyncInfo(
                    on_wait=keep,
                    on_update=list(si.on_update) if si.on_update else [])
                changed = True
            out.append(inst)
        if changed:
            blk.instructions = out


def _build_nc(sim_gelu=False, split=True):
    from contextlib import ExitStack
    import concourse.bass as bass
    import concourse.tile as tile
    import concourse.mybir as mybir

    _patch_tail_drain()

    fp32 = mybir.dt.float32
    bf16 = mybir.dt.bfloat16
    i32 = mybir.dt.int32
    fp8 = mybir.dt.float8e4
    AF = mybir.ActivationFunctionType
    OP = mybir.AluOpType

    u8 = mybir.dt.uint8

    nc = bass.Bass("TRN2", target_bir_lowering=False, num_devices=N_CORES)

    xp = nc.dram_tensor("xp", [T, DIM], bf16, kind="ExternalInput")
    # adj shipped bit-packed (bit-plane layout: byte k bit j <-> col j*128+k).
    # adjtp: transposed adj [h, t, s-planes] for the GCN (lhsT tiles directly);
    # adjmp: mask-layout rows [(sb,sc), (h,i), t-planes] for the softmax mask.
    adjtp = nc.dram_tensor("adjtp", [H, T, 128], u8, kind="ExternalInput")
    adjmp = nc.dram_tensor("adjmp", [32, 128, 128], u8, kind="ExternalInput")
    wBD = nc.dram_tensor("wBD", [DIM, 3 * DIM], bf16, kind="ExternalInput")
    wq = nc.dram_tensor("wq", [DIM, DIM], bf16, kind="ExternalInput")
    wk = nc.dram_tensor("wk", [DIM, DIM], bf16, kind="ExternalInput")
    wv = nc.dram_tensor("wv", [DIM, DIM], bf16, kind="ExternalInput")
    wkf = nc.dram_tensor("wkf", [2 * DIM, H], bf16, kind="ExternalInput")
    wkf2 = nc.dram_tensor("wkf2", [H, T], bf16, kind="ExternalInput")
    spD = nc.dram_tensor("spD", [H, T], fp32, kind="ExternalInput")
    bdrand = nc.dram_tensor("bdrand", [128, 128], bf16, kind="ExternalInput")
    eexp = nc.dram_tensor("eexp", [H, 2 * 128], bf16, kind="ExternalInput")
    identb = nc.dram_tensor("identb", [128, 128], bf16, kind="ExternalInput")
    identf = nc.dram_tensor("identf", [128, 128], fp32, kind="ExternalInput")
    outD = nc.dram_tensor("out", [HALF, DIM], fp32, kind="ExternalOutput")

    with tile.TileContext(nc) as tc, ExitStack() as ctx:
        # ---- long-lived pools
        wpool = ctx.enter_context(tc.tile_pool(name="weights", bufs=1))
        xpool = ctx.enter_context(tc.tile_pool(name="xq", bufs=1))
        npool = ctx.enter_context(tc.tile_pool(name="nmask", bufs=1))
        dpool = ctx.enter_context(tc.tile_pool(name="ddiag", bufs=1))
        ldpool = ctx.enter_context(tc.tile_pool(name="loads", bufs=2))
        tpool = ctx.enter_context(tc.tile_pool(name="tmp", bufs=3))

        # ---- constants / weights
        identB = wpool.tile([128, 128], bf16, tag="identB")
        identF = wpool.tile([128, 128], fp32, tag="identF")
        nc.sync.dma_start(identB[:], identb[:])
        nc.sync.dma_start(identF[:], identf[:])
        wBDs = [wpool.tile([128, 3 * DIM], bf16, tag=f"wBD{g}", name=f"wBD{g}")
                for g in range(2)]
        wqs = [wpool.tile([128, DIM], bf16, tag=f"wqs{g}", name=f"wqs{g}")
               for g in range(2)]
        wks = [wpool.tile([128, DIM], bf16, tag=f"wks{g}", name=f"wks{g}")
               for g in range(2)]
        wvs = [wpool.tile([128, DIM], bf16, tag=f"wvs{g}", name=f"wvs{g}")
               for g in range(2)]
        for g in range(2):
            nc.sync.dma_start(wBDs[g][:], wBD[g * 128:(g + 1) * 128, :])
            nc.sync.dma_start(wqs[g][:], wq[g * 128:(g + 1) * 128, :])
            nc.sync.dma_start(wks[g][:], wk[g * 128:(g + 1) * 128, :])
            nc.sync.dma_start(wvs[g][:], wv[g * 128:(g + 1) * 128, :])
        wkfs = [wpool.tile([128, H], bf16, tag=f"wkfs{g}", name=f"wkfs{g}")
                for g in range(4)]
        for g in range(4):
            nc.sync.dma_start(wkfs[g][:], wkf[g * 128:(g + 1) * 128, :])
        wkf2s = wpool.tile([H, T], bf16, tag="wkf2")
        nc.sync.dma_start(wkf2s[:], wkf2[:])
        spDs = wpool.tile([H, T], fp32, tag="spD")
        nc.sync.dma_start(spDs[:], spD[:])
        bdrs = wpool.tile([128, 128], bf16, tag="bdr")
        nc.sync.dma_start(bdrs[:], bdrand[:])
        eexps = wpool.tile([H, 256], bf16, tag="eexp")
        nc.sync.dma_start(eexps[:], eexp[:])

        xqkv = [xpool.tile([128, 3 * DIM], bf16, tag=f"xqkv{tb}",
                           name=f"xqkv{tb}") for tb in range(8)]
        vT = [xpool.tile([128, HALF], bf16, tag=f"vT{g}", name=f"vT{g}")
              for g in range(2)]
        ntiles = [npool.tile([128, T], bf16, tag=f"n{i}", name=f"n{i}")
                  for i in range(32)]
        svall = dpool.tile([128, 32], fp32, tag="svall")

        # q'/k' packed 3 heads per tile at bases {0,32,64}
        q1s = [xpool.tile([128, T], bf16, tag=f"q1s{j}", name=f"q1s{j}")
               for j in range(3)]
        k1s = [xpool.tile([128, T], bf16, tag=f"k1s{j}", name=f"k1s{j}")
               for j in range(3)]

        def q1h(h):
            return q1s[h // 3][(h % 3) * 32:(h % 3 + 1) * 32, :]

        def k1h(h):
            return k1s[h // 3][(h % 3) * 32:(h % 3 + 1) * 32, :]

        pkpool = ctx.enter_context(tc.tile_pool(name="pk", bufs=2))
        pmpool = ctx.enter_context(tc.tile_pool(name="pm", bufs=2))

        with tc.tile_pool(name="pmid", bufs=1) as mpool:
            qgT = [mpool.tile([128, T], bf16, tag=f"qgT{g}", name=f"qgT{g}")
                   for g in range(2)]
            kgT = [mpool.tile([128, T], bf16, tag=f"kgT{g}", name=f"kgT{g}")
                   for g in range(2)]
            qT = [mpool.tile([128, T], bf16, tag=f"qT{g}", name=f"qT{g}")
                  for g in range(2)]
            kT = [mpool.tile([128, T], bf16, tag=f"kT{g}", name=f"kT{g}")
                  for g in range(2)]
            Rexp = [mpool.tile([128, T], bf16, tag=f"Rexp{g}", name=f"Rexp{g}")
                    for g in range(2)]
            R0T = mpool.tile([H, T], bf16, tag="R0T")
            Rb = mpool.tile([H, T], bf16, tag="Rb")

            # ---- phase 0: x -> xT bf16
            with tc.tile_pool(name="p0", bufs=1) as p0, \
                 tc.tile_pool(name="ps0", bufs=2, space="PSUM") as ps0:
                xT = [p0.tile([128, T], bf16, tag=f"xT{g}", name=f"xT{g}")
                      for g in range(2)]
                for tb in range(8):
                    xb = tpool.tile([128, DIM], bf16, tag="xb", bufs=2)
                    nc.sync.dma_start(xb[:], xp[tb * 128:(tb + 1) * 128, :])
                    for g in range(2):
                        pt = ps0.tile([128, 128], bf16, tag="xTp")
                        nc.tensor.transpose(pt[:], xb[:, g * 128:(g + 1) * 128],
                                            identB[:])
                        nc.scalar.copy(xT[g][:, tb * 128:(tb + 1) * 128], pt[:])

                # ---- phase 1: xqkv
                for tb in range(8):
                    pt = ps0.tile([128, 3 * DIM], fp32, tag="xqkvp")
                    for g in range(2):
                        lhs = xT[g][:, tb * 128:(tb + 1) * 128]
                        nc.tensor.matmul(pt[:, 0:512], lhs, wBDs[g][:, 0:512],
                                         start=(g == 0), stop=(g == 1))
                        nc.tensor.matmul(pt[:, 512:768], lhs,
                                         wBDs[g][:, 512:768],
                                         start=(g == 0), stop=(g == 1))
                    nc.scalar.copy(xqkv[tb][:], pt[:])

            # ---- phase 2: GCN. adjT arrives bit-packed; unpack on DVE
            # ((pk >> j) & 1 per bit-plane, then u8 -> bf16 copy) straight
            # into the transposed layout the matmul wants — no PE transpose.
            with tc.tile_pool(name="ps2", bufs=2, space="PSUM") as ps2, \
                 tc.tile_pool(name="adjt", bufs=2) as apool:
                for h in range(8):
                    hg, hh = h // 4, h % 4
                    # one PSUM bank holds all 8 sb-accumulators as col slices
                    pacc = ps2.tile([128, 8 * 2 * D], fp32, tag="pacc")
                    for tb in range(8):
                        pkh = ldpool.tile([128, 128], u8, tag="pkh")
                        nc.sync.dma_start(
                            pkh[:], adjtp[h, tb * 128:(tb + 1) * 128, :])
                        adjTt = apool.tile([128, T], bf16, tag="adjTt")
                        unp = tpool.tile([128, T], u8, tag="unp")
                        for j in range(8):
                            nc.vector.tensor_scalar(
                                unp[:, j * 128:(j + 1) * 128], pkh[:], j, 1,
                                OP.logical_shift_right, OP.bitwise_and)
                        nc.vector.tensor_copy(adjTt[:], unp[:])
                        rhs = xqkv[tb][:].rearrange(
                            "p (w c) -> p w c", w=3)[:, 0:2, h * D:(h + 1) * D]
                        for sb in range(8):
                            # start=True clears has_written for the WHOLE
                            # bank, so only the very first matmul into this
                            # bank may carry it; later regions' first writes
                            # overwrite-on-clear and then accumulate.
                            nc.tensor.matmul(
                                pacc[:, sb * 2 * D:(sb + 1) * 2 * D],
                                adjTt[:, sb * 128:(sb + 1) * 128],
                                rhs, start=(tb == 0 and sb == 0),
                                stop=(tb == 7))
                    qkg = tpool.tile([128, 8 * 2 * D], bf16, tag="qkg")
                    nc.scalar.activation(qkg[:], pacc[:], AF.Relu)
                    for sb in range(8):
                        pgT = ps2.tile([32, 128], bf16, tag="pgT")
                        pkT = ps2.tile([32, 128], bf16, tag="pkT")
                        nc.tensor.transpose(pgT[:], qkg[:, sb * 2 * D:
                                                        sb * 2 * D + D],
                                            identB[:])
                        nc.tensor.transpose(pkT[:], qkg[:, sb * 2 * D + D:
                                                        (sb + 1) * 2 * D],
                                            identB[:])
                        nc.scalar.copy(
                            qgT[hg][hh * 32:(hh + 1) * 32,
                                    sb * 128:(sb + 1) * 128], pgT[:])
                        nc.scalar.copy(
                            kgT[hg][hh * 32:(hh + 1) * 32,
                                    sb * 128:(sb + 1) * 128], pkT[:])

            # ---- phase 3: projections (stacked transposed)
            with tc.tile_pool(name="ps3", bufs=2, space="PSUM") as ps3:
                for (src, w, dst) in ((qgT, wqs, qT), (kgT, wks, kT)):
                    for ob in range(2):
                        pt = ps3.tile([128, T], fp32, tag="p4big")
                        for g in range(2):
                            for nh in range(2):
                                nc.tensor.matmul(
                                    pt[:, nh * 512:(nh + 1) * 512],
                                    w[g][:, ob * 128:(ob + 1) * 128],
                                    src[g][:, nh * 512:(nh + 1) * 512],
                                    start=(g == 0), stop=(g == 1))
                        nc.scalar.copy(dst[ob][:], pt[:])

                # ---- phase 4: R chain
                pR0 = ps3.tile([H, T], fp32, tag="p4big")
                srcs = [qT[0], qT[1], kT[0], kT[1]]
                for w in range(4):
                    for nh in range(2):
                        nc.tensor.matmul(pR0[:, nh * 512:(nh + 1) * 512],
                                         wkfs[w][:],
                                         srcs[w][:, nh * 512:(nh + 1) * 512],
                                         start=(w == 0), stop=(w == 3))
                # gelu(x) ~= x * sigmoid(1.702 x) (same path on sim and hw)
                sgR0 = tpool.tile([H, T], fp32, tag="sgR0", bufs=1)
                nc.scalar.activation(sgR0[:], pR0[:], AF.Sigmoid, scale=1.702)
                nc.vector.tensor_tensor(R0T[:], pR0[:], sgR0[:], OP.mult)
                pG = ps3.tile([H, H], fp32, tag="pG", bufs=1)
                for tb in range(8):
                    pr = ps3.tile([128, H], bf16, tag="pR0T")
                    nc.tensor.transpose(pr[:], R0T[:, tb * 128:(tb + 1) * 128],
                                        identB[0:8, 0:8])
                    r0 = tpool.tile([128, H], bf16, tag="r0")
                    nc.scalar.copy(r0[:], pr[:])
                    nc.tensor.matmul(pG[:], r0[:], r0[:],
                                     start=(tb == 0), stop=(tb == 7))
                gram = tpool.tile([H, H], bf16, tag="gram")
                nc.scalar.copy(gram[:], pG[:])
                pRp = ps3.tile([H, T], fp32, tag="p4big")
                for nh in range(2):
                    nc.tensor.matmul(pRp[:, nh * 512:(nh + 1) * 512], gram[:],
                                     wkf2s[:, nh * 512:(nh + 1) * 512],
                                     start=True, stop=True)
                spDi = tpool.tile([H, T], fp32, tag="spDi", bufs=1)
                nc.vector.reciprocal(spDi[:], spDs[:])
                rpre = tpool.tile([H, T], fp32, tag="rpre", bufs=1)
                nc.vector.tensor_tensor(rpre[:], pRp[:], spDi[:], OP.mult)
                nc.scalar.activation(Rb[:], rpre[:], AF.Sigmoid)
                for g in range(2):
                    pt = ps3.tile([128, T], fp32, tag="p4big")
                    for nh in range(2):
                        nc.tensor.matmul(pt[:, nh * 512:(nh + 1) * 512],
                                         eexps[:, g * 128:(g + 1) * 128],
                                         Rb[:, nh * 512:(nh + 1) * 512],
                                         start=True, stop=True)
                    nc.scalar.copy(Rexp[g][:], pt[:])

            # ---- phase 5: q' = qT * Rexp * scale, k' = kT * Rexp (3-packed)
            scale = float(DIM) ** (-0.5)
            for h in range(8):
                g, hh = h // 4, h % 4
                sl = slice(hh * 32, (hh + 1) * 32)
                nc.vector.scalar_tensor_tensor(q1h(h), qT[g][sl, :], scale,
                                               Rexp[g][sl, :], OP.mult, OP.mult)
                nc.vector.tensor_tensor(k1h(h), kT[g][sl, :], Rexp[g][sl, :],
                                        OP.mult)

        # ---- phase 6: logits + mixing + masked softmax (my 4 row-blocks)
        with tc.tile_pool(name="ps6", bufs=2, space="PSUM") as ps6, \
             tc.tile_pool(name="ps6b", bufs=2, space="PSUM") as ps6b:
            for sb in range(4):
                packed = pkpool.tile([128, 8 * T], bf16, tag="packed")
                # decoy write: absorbs the packed-slot WAR (PE) dep so the
                # real repack DMAs stay under the 2-sync-wait walrus limit
                nc.sync.dma_start(packed[0:1, 0:2], identB[0:1, 0:2])
                for h in range(8):
                    pa = ps6.tile([128, T], fp32, tag="pattn")
                    lhs = q1h(h)[:, sb * 128:(sb + 1) * 128]
                    for nh in range(2):
                        nc.tensor.matmul(pa[:, nh * 512:(nh + 1) * 512], lhs,
                                         k1h(h)[:, nh * 512:(nh + 1) * 512],
                                         start=True, stop=True)
                    pmt = pmpool.tile([128, T], bf16, tag="pmt")
                    nc.vector.tensor_scalar_mul(pmt[:], pa[:], 0.01)
                    pm = pmpool.tile([128, T], bf16, tag="pm")
                    nc.vector.tensor_tensor(pm[:], pa[:], pmt[:], OP.max)
                    for sc in range(8):
                        nc.sync.dma_start(
                            packed[16 * h:16 * h + 16, sc * T:(sc + 1) * T],
                            pm[sc * 16:(sc + 1) * 16, :])
                for sc in range(8):
                    pkm = ldpool.tile([128, 128], u8, tag="pkm")
                    nc.sync.dma_start(pkm[:], adjmp[sb * 8 + sc])
                    amu = tpool.tile([128, T], u8, tag="amu")
                    for j in range(8):
                        nc.vector.tensor_scalar(
                            amu[:, j * 128:(j + 1) * 128], pkm[:], j, 1,
                            OP.logical_shift_right, OP.bitwise_and)
                    am = ldpool.tile([128, T], bf16, tag="am")
                    nc.vector.tensor_copy(am[:], amu[:])
                    ssum = [None, None]
                    ntile = ntiles[sb * 8 + sc]
                    for nh in range(2):
                        pmix = ps6b.tile([128, 512], fp32, tag="pmix")
                        nc.tensor.matmul(pmix[:], bdrs[:],
                                         packed[:, sc * T + nh * 512:
                                                sc * T + (nh + 1) * 512],
                                         start=True, stop=True)
                        et = tpool.tile([128, 512], bf16, tag="et")
                        nc.scalar.activation(et[:], pmix[:], AF.Exp)
                        st = tpool.tile([128, 1], fp32, tag=f"st{nh}",
                                        name=f"st{nh}")
                        nc.vector.scalar_tensor_tensor(
                            ntile[:, nh * 512:(nh + 1) * 512], et[:], 1.0,
                            am[:, nh * 512:(nh + 1) * 512], OP.mult, OP.mult,
                            accum_out=st[:])
                        ssum[nh] = st
                    stot = tpool.tile([128, 1], fp32, tag="stot")
                    nc.vector.tensor_tensor(stot[:], ssum[0][:], ssum[1][:],
                                            OP.add)
                    nc.vector.reciprocal(
                        svall[:, sb * 8 + sc:sb * 8 + sc + 1], stot[:])

        # ---- phase 7: v = relu(softmax @ xv)
        with tc.tile_pool(name="nt", bufs=1) as ntpool, \
             tc.tile_pool(name="psv", bufs=1, space="PSUM") as psv, \
             tc.tile_pool(name="ps7", bufs=2, space="PSUM") as ps7:
            pvT = [psv.tile([64, HALF], fp32, tag=f"pvT{j}", name=f"pvT{j}")
                   for j in range(4)]
            for sb in range(4):
                dsc = [ntpool.tile([128, 128], bf16, tag=f"dsc{sc}",
                                   name=f"dsc{sc}") for sc in range(8)]
                for sc in range(8):
                    nc.vector.tensor_scalar_mul(
                        dsc[sc][:], identB[:],
                        svall[:, sb * 8 + sc:sb * 8 + sc + 1])
                ntb = [ntpool.tile([128, 8 * 128], bf16, tag=f"ntb{tb}",
                                   name=f"ntb{tb}") for tb in range(8)]
                for tb in range(8):
                    for sc in range(8):
                        pnT = ps7.tile([128, 128], fp32, tag="pnT")
                        nc.tensor.matmul(
                            pnT[:],
                            ntiles[sb * 8 + sc][:, tb * 128:(tb + 1) * 128],
                            dsc[sc][:], start=True, stop=True)
                        nc.vector.tensor_copy(
                            ntb[tb][:, sc * 128:(sc + 1) * 128], pnT[:])
                for l in range(8):
                    for tb in range(8):
                        lhs = xqkv[tb][:, 2 * DIM + l * D:2 * DIM + (l + 1) * D]
                        rhs = ntb[tb][:].rearrange(
                            "p (sc c) -> p sc c", sc=8)[:, :,
                                                        l * 16:(l + 1) * 16]
                        nc.tensor.matmul(
                            pvT[l // 2][(l % 2) * 32:(l % 2 + 1) * 32,
                                        sb * 128:(sb + 1) * 128],
                            lhs, rhs, start=(tb == 0), stop=(tb == 7))
            for l in range(8):
                nc.scalar.activation(
                    vT[l // 4][(l % 4) * 32:(l % 4 + 1) * 32, :],
                    pvT[l // 2][(l % 2) * 32:(l % 2 + 1) * 32, :], AF.Relu)

        # ---- phase 8: out = gelu(v @ Wv), transpose, store
        with tc.tile_pool(name="ps8", bufs=2, space="PSUM") as ps8:
            for ob in range(2):
                po = ps8.tile([128, HALF], fp32, tag="po")
                for g in range(2):
                    nc.tensor.matmul(po[:], wvs[g][:, ob * 128:(ob + 1) * 128],
                                     vT[g][:], start=(g == 0), stop=(g == 1))
                oT = tpool.tile([128, HALF], fp32, tag="oT", bufs=1)
                if sim_gelu:
                    sg = tpool.tile([128, HALF], fp32, tag="sg", bufs=1)
                    nc.scalar.activation(sg[:], po[:], AF.Sigmoid, scale=1.702)
                    nc.vector.tensor_tensor(oT[:], po[:], sg[:], OP.mult)
                else:
                    nc.scalar.activation(oT[:], po[:], AF.Gelu)
                for sb in range(4):
                    pf = ps8.tile([128, 128], fp32, tag="pf")
                    nc.tensor.transpose(pf[:], oT[:, sb * 128:(sb + 1) * 128],
                                        identF[:])
                    ofin = tpool.tile([128, 128], fp32, tag="ofin")
                    nc.scalar.copy(ofin[:], pf[:])
                    nc.sync.dma_start(
                        outD[sb * 128:(sb + 1) * 128,
                             ob * 128:(ob + 1) * 128], ofin[:])
    if split:
        _split_waits(nc, mybir)
    return nc


def _prep_inputs(x, adj, Wq_g, Wk_g, Wv_g, Wq, Wk, Wv, Wkf, Wkf2, sparse_D,
                 randomatrix):
    import ml_dtypes
    bf = ml_dtypes.bfloat16

    def bd3(*Ws):
        out = np.zeros((DIM, 3 * DIM), np.float32)
        for wi, W in enumerate(Ws):
            for h in range(H):
                out[h * D:(h + 1) * D, wi * DIM + h * D:wi * DIM + (h + 1) * D] = W
        return out

    wBD = bd3(Wq_g, Wk_g, Wv_g).astype(bf)
    bdrand = np.zeros((128, 128), np.float32)
    for h in range(H):
        for l in range(H):
            for i in range(16):
                bdrand[h * 16 + i, l * 16 + i] = randomatrix[l, h]
    bdrand = bdrand.astype(bf)
    eexp = np.zeros((H, 256), np.float32)
    for g in range(2):
        for m in range(128):
            h = g * 4 + m // 32
            eexp[h, g * 128 + m] = 1.0
    eexp = eexp.astype(bf)
    ident = np.eye(128, dtype=np.float32)

    shared = dict(
        wBD=wBD, wq=Wq.astype(bf), wk=Wk.astype(bf), wv=Wv.astype(bf),
        wkf=Wkf.astype(bf), bdrand=bdrand, eexp=eexp,
        identb=ident.astype(bf), identf=ident.astype(np.float32),
    )
    # wkf2 / sparse_D are indexed by absolute token position -> their columns
    # must follow the per-core permutation (half-block swap).
    wkf2b = Wkf2.astype(bf)
    wkf2sw = np.concatenate([wkf2b[:, HALF:], wkf2b[:, :HALF]], axis=1)
    spDf = sparse_D.astype(np.float32)
    spDsw = np.concatenate([spDf[:, HALF:], spDf[:, :HALF]], axis=1)

    # Bit-pack adj once (bit-plane layout: byte k bit j <-> col j*128+k of
    # the unpacked row). The per-core permutation swaps 512-blocks in both
    # axes: a row block swap is a slice-concat; a column block swap moves
    # plane j to (j+4)%8, i.e. a nibble swap on the packed byte.
    au8 = adj.astype(np.uint8)
    aT = np.ascontiguousarray(au8.transpose(0, 1, 3, 2))     # [b,h,t,s]
    bitsT = aT.reshape(B, H, T, 8, 128)
    pk_gcnF = np.zeros((B, H, T, 128), np.uint8)
    bitsM = au8.reshape(B, H, T, 8, 128)
    pk_maskF = np.zeros((B, H, T, 128), np.uint8)
    for j in range(8):
        pk_gcnF |= bitsT[:, :, :, j, :] << j
        pk_maskF |= bitsM[:, :, :, j, :] << j

    def nibswap(a):
        return ((a >> 4) | (a << 4)).astype(np.uint8)

    in_maps = []
    for c in range(N_CORES):
        b, half = c // 2, c % 2
        s0 = half * HALF
        pk = pk_gcnF[b]
        if half:
            pk = nibswap(np.concatenate([pk[:, HALF:, :], pk[:, :HALF, :]],
                                        axis=1))
        pm_ = pk_maskF[b][:, s0:s0 + HALF, :]
        if half:
            pm_ = nibswap(pm_)
        adjmp = np.ascontiguousarray(
            pm_.reshape(H, 4, 8, 16, 128).transpose(1, 2, 0, 3, 4)
            .reshape(32, 128, 128))
        xpm = np.concatenate([x[b][s0:s0 + HALF],
                              x[b][HALF - s0:HALF - s0 + HALF]]).astype(bf)
        m = dict(shared)
        m.update(xp=xpm, adjtp=np.ascontiguousarray(pk), adjmp=adjmp,
                 wkf2=wkf2sw if half else wkf2b, spD=spDsw if half else spDf)
        in_maps.append(m)
    return in_maps


def kernel(x, adj, Wq_g, Wk_g, Wv_g, Wq, Wk, Wv, Wkf, Wkf2, sparse_D,
           randomatrix, label):
    from concourse.bass_utils import run_bass_kernel_spmd

    if "nc" not in _CACHE:
        _CACHE["nc"] = _build_nc(sim_gelu=True)
    nc = _CACHE["nc"]

    in_maps = _prep_inputs(np.asarray(x), np.asarray(adj), np.asarray(Wq_g),
                           np.asarray(Wk_g), np.asarray(Wv_g), np.asarray(Wq),
                           np.asarray(Wk), np.asarray(Wv), np.asarray(Wkf),
                           np.asarray(Wkf2), np.asarray(sparse_D),
                           np.asarray(randomatrix))
    res = run_bass_kernel_spmd(nc, in_maps, core_ids=list(range(N_CORES)))
    _CACHE["last_result"] = res

    out = np.empty((B, T, DIM), np.float32)
    for c in range(N_CORES):
        b, half = c // 2, c % 2
        out[b, half * HALF:(half + 1) * HALF] = res.results[c]["out"]
    return out

